# revision 25
# baseline (speedup 1.0000x reference)
"""MoE MLP (dense all-experts routing) Trainium2 Bass kernel.

Math (reference):
    g   = softmax(x @ gate_w + gate_b)            # [N, E]
    h   = relu(einsum("nd,edh->neh", x, w1) + b1) # [N, E, H]
    out = einsum("neh,ehd,ne->nd", h, w2, g)      # [N, D]

With E=64, H=16 (E*H = 1024 = D) this is two dense [1024,1024] matmuls plus a
small gate matmul.  Expert-hidden axis is reordered as eh' = h*64 + e
(h-major) so the gating multiply is a plain elementwise multiply of every
128-row tile of h^T by one shared [128, tok] tile of duplicated gate probs.

Layouts on device are feature-major (x^T, h^T, out^T); tokens are the matmul
moving (free) dimension.  Sharding: data-parallel over tokens, 4096 per core,
8 cores, no collectives.  Matmuls run in bf16 with fp32 PSUM accumulation.

Perf notes (vs the 255.5us previous version; PE floor for this decomposition
is ~225us of pure matmul streaming):
  - Gate logits col-tiled: even k -> partitions 0:64, odd k -> 64:128,
    concurrent pairs in the two array halves (4 PE slots per tile for 8 k
    chunks).  The two halves are combined WITHOUT a merge matmul:
    exp(a+b+gb) = exp(a)*exp(b+gb), so one scalar exp over all 128
    partitions (bias [0;gb]) followed by a partition-shift copy of the
    upper half and an elementwise multiply+dup (DVE reads may come from any
    partition window; writes can't straddle the 64-partition boundary).
    Saves one PE slot per tile and the lgsb round-trip.
  - Head: misc consts ride the fast sync ring FIRST (they gate the gate
    matmuls); x tile 0 is split 4+2 chunks on sync for progressive arrival;
    xb0 rides scalar ahead of w2 so gate pair 3 isn't stuck behind xa0.
    Warmup weights are memset on gpsimd (done ~6.5us) so the HAM warmup
    matmuls start right at PE preamble end (~7.2us).
  - 14 N=512 warmup matmuls cover the initial DMA wait and un-throttle the
    PE clock gate (4/8 -> 8/8) so real work starts warm.
  - Tail: the last tile's outputs avoid the gpsimd software-DGE queue
    (its end-of-kernel DRAIN polls ~3us behind the last packet); they ride
    sync, with the final block split sync/scalar so the teardown barrier
    isn't gated on a late software-queue drain.
  - DMA queues share HBM per-queue round-robin; the sync ring (~190GB/s)
    carries misc + x + w1 pairs 1-3 in exact consumption order; w1 pair 0
    rides the otherwise-idle gpsimd ring; w2 pairs ride scalar behind xb.
    The x pool is single-buffered so tile t+1's x DMA stays behind tile
    t's last read instead of being hoisted above the weight stream.
"""

import numpy as np
import ml_dtypes

N, D, E, H = 32768, 1024, 64, 16
EH = E * H  # 1024
NCORES = 8
NTOK = N // NCORES  # tokens per core
TT = 512            # token tile (one PSUM bank of fp32)
KC = D // 128       # 8 contraction chunks for D
MC = EH // 128      # 8 output row-tiles for EH (and for D in stage 2)
WARMUP = 13         # N=512 warmup matmuls covering the initial DMA wait

_CACHE = {}


def build(n_tok=NTOK):
    """Build + compile the per-core Bass kernel for n_tok tokens."""
    import concourse.bass as bass
    import concourse.mybir as mybir
    import concourse.tile as tile
    from concourse import bacc

    f32 = mybir.dt.float32
    bf16 = mybir.dt.bfloat16
    AF = mybir.ActivationFunctionType
    nt = n_tok // TT
    assert n_tok % TT == 0

    nc = bacc.Bacc("TRN2", target_bir_lowering=False, debug=False)

    KA = 6  # x-tile k-chunks in the first (big) half; KC-KA in the second
    xda = nc.dram_tensor("xda", [nt, 128, KA, TT], bf16, kind="ExternalInput")
    xdb = nc.dram_tensor("xdb", [nt, 128, KC - KA, TT], bf16, kind="ExternalInput")
    # all small consts packed into one tensor = one DMA:
    # cols [0:512] gate_w (k-major), [512] exp bias ([0]*64 || gate_b),
    # [513:521] b1 m-columns -- biases in bf16
    miscd = nc.dram_tensor("miscd", [128, 524], bf16, kind="ExternalInput")
    # weights in m-block PAIRS so each DMA is 4KB/partition (4KB descriptors
    # get full ring throughput; 2KB ones don't)
    w1d = nc.dram_tensor("w1d", [MC // 2, 128, 2, KC, 128], bf16, kind="ExternalInput")
    w2d = nc.dram_tensor("w2d", [MC // 2, 128, 2, KC, 128], bf16, kind="ExternalInput")
    outT = nc.dram_tensor("outT", [nt, MC, 128, TT], bf16, kind="ExternalOutput")

    with tile.TileContext(nc) as tc:
        with (
            tc.tile_pool(name="consts", bufs=1) as consts,
            tc.tile_pool(name="xp", bufs=1) as xp,
            tc.tile_pool(name="sp", bufs=2) as sp,
            tc.tile_pool(name="hp", bufs=4) as hp,
            tc.tile_pool(name="ps1", bufs=1, space=bass.MemorySpace.PSUM) as ps1,
            tc.tile_pool(name="ps2", bufs=3, space=bass.MemorySpace.PSUM) as ps2,
            tc.tile_pool(name="ps3", bufs=2, space=bass.MemorySpace.PSUM) as ps3,
        ):
            # --- delivery plan: ring FIFO is the priority mechanism.
            # sync ring: misc -> xa0 (split 4+2) -> w1 pairs 1-3 -> xa tiles
            #   and odd outputs.
            # scalar ring: xb0 -> w2 pairs -> xb tiles.
            # gpsimd ring: w1 pair 0, then even outputs (none for the last
            #   tile, so its software-DGE drain isn't on the tail path). ---
            # halfZ of 0.5 contracts both duplicated e2 halves over all 128
            # partitions (= sum over the 64 experts); full-row LDWEIGHTS
            # keeps pull-ahead pipelining (a 64-row Z would block it)
            halfZ = consts.tile([128, 128], bf16)
            nc.gpsimd.memset(halfZ[:], 0.5)
            # warmup weights memset on gpsimd: ready before the PE preamble
            # ends, unlike a vector memset (vector preamble ends ~7.4us)
            wsc = consts.tile([128, TT], bf16, tag="wsc")
            nc.gpsimd.memset(wsc[:], 0.0)

            # The head is HBM-bandwidth-bound (~300GB/s aggregate from
            # ~8.5us); delivery order = first-use order:
            #   sync (fast ~200GB/s ramp):  xa0 -> w1 m0 -> w1 pairs 1-3
            #   gpsimd (software DGE):      w1 m1
            #   scalar (slow ~80GB/s ramp): misc -> xb0 -> w2 pairs
            # CRITICAL: any PE idle gap over ~1us in the first ~25us makes
            # the HAM clock-gate RE-THROTTLE to 4/8 for a 3.4-6.8us window
            # (half-rate matmuls); the tile-0 schedule below keeps the PE
            # continuously fed (warmup fills bridge the sub-us waits).
            misc = consts.tile([128, 524], bf16)
            nc.scalar.dma_start(out=misc[:], in_=miscd[:])

            xa0 = xp.tile([128, KA, TT], bf16, tag="xa")
            nc.sync.dma_start(out=xa0[:], in_=xda[0])
            xb0 = xp.tile([128, KC - KA, TT], bf16, tag="xb")
            nc.scalar.dma_start(out=xb0[:], in_=xdb[0])

            w1_sb = consts.tile([128, MC, KC, 128], bf16)
            w2_sb = consts.tile([128, MC, KC, 128], bf16)
            nc.sync.dma_start(out=w1_sb[:, 0], in_=w1d[0][:, 0])
            nc.gpsimd.dma_start(out=w1_sb[:, 1], in_=w1d[0][:, 1])
            for p in range(1, MC // 2):
                nc.sync.dma_start(out=w1_sb[:, 2 * p:2 * p + 2], in_=w1d[p])
            for p in range(MC // 2):
                nc.scalar.dma_start(out=w2_sb[:, 2 * p:2 * p + 2], in_=w2d[p])

            def gw_k(k):
                return misc[:, k * 64:(k + 1) * 64]

            expbias = misc[:, 512:513]

            def b1_m(m):
                return misc[:, 513 + m:514 + m]

            # HAM warmup: dummy matmuls on a zeroed scratch tile fill the
            # initial weight/x DMA wait and un-throttle the PE clock gate
            # (4/8 -> 8/8) before real work arrives.  They use the zb slot
            # (first real use: the Z matmul, well after the gate) so the
            # gate's lg group has no dependency on them.
            wps = ps1.tile([128, TT], f32, tag="zb")
            for i in range(WARMUP):
                nc.tensor.matmul(wps[:], wsc[:, 0:128], wsc[:],
                                 start=(i == 0), stop=(i == WARMUP - 1))

            def wfill():
                # single filler matmul: bridges a sub-us data wait in the
                # tile-0 schedule so the PE never idles (idle > ~1us makes
                # the HAM clock-gate re-throttle to half rate)
                nc.tensor.matmul(wps[:], wsc[:, 0:128], wsc[:],
                                 start=True, stop=True)

            for t in range(nt):
                # x tile in two k-halves on separate HWDGE rings so each
                # tile lands as early as possible
                if t == 0:
                    xa, xb = xa0, xb0
                else:
                    xa = xp.tile([128, KA, TT], bf16, tag="xa")
                    nc.sync.dma_start(out=xa[:], in_=xda[t])
                    xb = xp.tile([128, KC - KA, TT], bf16, tag="xb")
                    nc.scalar.dma_start(out=xb[:], in_=xdb[t])

                def xk(k):
                    return xa[:, k, :] if k < KA else xb[:, k - KA, :]

                # --- gate logits, col-tiled: even k chunks accumulate into
                # partitions 0:64 (array cols 0-63), odd into 64:128 --
                # consecutive pairs run concurrently in the array halves.
                hg = sp.tile([128, MC, TT], bf16, tag="hg")
                h_tiles = []

                def stage1_mm(m, ks=range(KC), hps=None):
                    if hps is None:
                        hps = ps2.tile([128, TT], f32, tag="hps")
                    for k in ks:
                        nc.tensor.matmul(
                            hps[:], w1_sb[:, m, k, :], xk(k),
                            start=(k == 0), stop=(k == KC - 1),
                            skip_group_check=True,
                        )
                    return hps

                def stage1_act(m, hps):
                    h = hp.tile([128, TT], bf16, tag="h")
                    nc.scalar.activation(
                        h[:], hps[:], AF.Relu, bias=b1_m(m), scale=1.0
                    )
                    h_tiles.append((m, h))

                def stage1(m):
                    stage1_act(m, stage1_mm(m))

                # two lg banks alternate per tile so the next tile's gate
                # group start doesn't wait on this tile's exp read (WAW)
                lg = ps1.tile([128, TT], f32, tag="lg", bufs=2)

                def gate_pair(j):
                    nc.tensor.matmul(
                        lg[0:64, :], gw_k(2 * j), xk(2 * j),
                        start=(j == 0), stop=(j == 3), skip_group_check=True,
                    )
                    nc.tensor.matmul(
                        lg[64:128, :], gw_k(2 * j + 1), xk(2 * j + 1),
                        start=(j == 0), stop=(j == 3), skip_group_check=True,
                    )

                if t == 0:
                    # tile 0 in data-arrival order, gap-free: pairs 0-2
                    # (misc+xa0 ~12.5us), m0-m2 xa chunks as w1 m-blocks
                    # land (sync m0 ~13.8, gpsimd m1 ~15.5, sync pair1
                    # ~16.4), pair 3 once xb0 lands on scalar (~17), then
                    # the deferred k6/k7 completions.  wfill() bridges the
                    # two sub-us waits.
                    gate_pair(0)
                    gate_pair(1)
                    gate_pair(2)
                    wfill()
                    hps0 = stage1_mm(0, range(KA))
                    wfill()
                    hps1 = stage1_mm(1, range(KA))
                    hps2 = stage1_mm(2, range(KA))
                    gate_pair(3)
                    stage1_mm(0, range(KA, KC), hps0)
                    stage1_mm(1, range(KA, KC), hps1)
                    stage1_mm(2, range(KA, KC), hps2)
                else:
                    for j in range(4):
                        gate_pair(j)
                    hps0 = stage1_mm(0)

                # duplicated gate probs without a merge matmul:
                # e2[p] = exp(a_p)*exp(b_p+gb) for p%64 halves; exp runs
                # straight off the lg PSUM, the upper half is shift-copied
                # down, multiplied, and the product shift-copied back up.
                e_all = sp.tile([128, TT], bf16, tag="eall")
                nc.scalar.activation(e_all[:], lg[:], AF.Exp,
                                     bias=expbias, scale=1.0)
                et = sp.tile([64, TT], bf16, tag="etmp")
                nc.vector.tensor_copy(et[:], e_all[64:128, :])
                e2 = sp.tile([128, TT], bf16, tag="e2")
                nc.vector.tensor_mul(e2[0:64, :], e_all[0:64, :], et[:])
                nc.vector.tensor_copy(e2[64:128, :], e2[0:64, :])
                stage1_act(0, hps0)

                if t == 0:
                    stage1_act(1, hps1)
                else:
                    stage1(1)

                # Z = sum of exp over the 64 experts, broadcast to all 128
                # partitions (0.5 * both duplicated halves), then g2 = e2/Z
                zb = ps1.tile([128, TT], f32, tag="zb")
                nc.tensor.matmul(zb[:], halfZ[:], e2[:], start=True, stop=True)

                if t == 0:
                    stage1_act(2, hps2)
                else:
                    stage1(2)

                rzb = sp.tile([128, TT], f32, tag="rzb")
                nc.vector.reciprocal_approx_fast(rzb[:], zb[:])
                g2 = sp.tile([128, TT], bf16, tag="g2")
                nc.vector.tensor_mul(g2[:], e2[:], rzb[:])

                for m, h in h_tiles:
                    nc.vector.tensor_mul(hg[:, m, :], h[:], g2[:])
                for m in range(3, MC):
                    stage1(m)
                    _, h = h_tiles[-1]
                    nc.vector.tensor_mul(hg[:, m, :], h[:], g2[:])

                # --- stage 2: out^T tiles ---
                last = t == nt - 1
                for m2 in range(MC):
                    ops = ps3.tile([128, TT], f32, tag="ops")
                    for k in range(MC):
                        nc.tensor.matmul(
                            ops[:], w2_sb[:, m2, k, :], hg[:, k, :],
                            start=(k == 0), stop=(k == MC - 1),
                        )
                    osb = hp.tile([128, TT], bf16, tag="osb")
                    if last and m2 == MC - 1:
                        # final output block: cast halves on two engines and
                        # DMA on two hardware rings in parallel so the tail
                        # drains sooner (gpsimd's software queue would add
                        # ~3us of end-of-kernel drain polling)
                        nc.vector.tensor_copy(osb[:, 0:TT // 2], ops[:, 0:TT // 2])
                        nc.scalar.copy(osb[:, TT // 2:], ops[:, TT // 2:])
                        nc.sync.dma_start(
                            out=outT[t, m2][:, 0:TT // 2], in_=osb[:, 0:TT // 2])
                        nc.scalar.dma_start(
                            out=outT[t, m2][:, TT // 2:], in_=osb[:, TT // 2:])
                        continue
                    nc.vector.tensor_copy(osb[:], ops[:])
                    # alternate output rings: halves the gpsimd ring load
                    # and lets the last tile's DMAs drain in parallel; the
                    # last tile stays entirely off the gpsimd ring
                    if m2 % 2 == 1 or last:
                        nc.sync.dma_start(out=outT[t, m2], in_=osb[:])
                    else:
                        nc.gpsimd.dma_start(out=outT[t, m2], in_=osb[:])

    nc.compile()
    return nc


def host_prep(x, gate_w, gate_b, w1, b1, w2):
    bf = ml_dtypes.bfloat16
    nt = NTOK // TT
    KA = 6
    xb = x.astype(bf)
    x_shards = []
    for c in range(NCORES):
        xt = (xb[c * NTOK:(c + 1) * NTOK]
              .reshape(nt, TT, KC, 128).transpose(0, 3, 2, 1))
        x_shards.append((np.ascontiguousarray(xt[:, :, :KA]),
                         np.ascontiguousarray(xt[:, :, KA:])))
    miscd = np.zeros((128, 524), dtype=bf)
    miscd[:, 0:512] = (
        gate_w.astype(bf).reshape(KC, 128, 64).transpose(1, 0, 2)
        .reshape(128, 512))
    # exp bias: upper half gets gate_b, lower half 0 (the product of the
    # two exp halves then carries exp(gate_b) exactly once)
    miscd[64:128, 512] = gate_b.astype(bf)
    # eh' = h*64 + e ordering
    miscd[:, 513:521] = b1.T.reshape(EH).astype(bf).reshape(MC, 128).T
    w1d = np.ascontiguousarray(
        w1.transpose(1, 2, 0).reshape(D, EH).astype(bf)
        .reshape(KC, 128, MC // 2, 2, 128).transpose(2, 1, 3, 0, 4))
    w2d = np.ascontiguousarray(
        w2.transpose(1, 0, 2).reshape(EH, D).astype(bf)
        .reshape(KC, 128, MC // 2, 2, 128).transpose(2, 1, 3, 0, 4))
    common = {"miscd": miscd, "w1d": w1d, "w2d": w2d}
    return x_shards, common


def kernel(x, gate_w, gate_b, w1, b1, w2, _trace=False):
    import concourse.bass_utils as bass_utils

    x = np.asarray(x, dtype=np.float32)
    gate_w = np.asarray(gate_w, dtype=np.float32)
    gate_b = np.asarray(gate_b, dtype=np.float32)
    w1 = np.asarray(w1, dtype=np.float32)
    b1 = np.asarray(b1, dtype=np.float32)
    w2 = np.asarray(w2, dtype=np.float32)

    if "nc" not in _CACHE:
        _CACHE["nc"] = build(NTOK)
    nc = _CACHE["nc"]

    x_shards, common = host_prep(x, gate_w, gate_b, w1, b1, w2)
    in_maps = [dict(common, xda=x_shards[c][0], xdb=x_shards[c][1])
               for c in range(NCORES)]
    try:
        res = bass_utils.run_bass_kernel_spmd(
            nc, in_maps, core_ids=list(range(NCORES)), trace=_trace
        )
    except Exception:
        # transient device states (e.g. NRT_EXEC_UNIT_UNRECOVERABLE after a
        # wedged prior run) usually clear after a pause; retry once
        import time
        time.sleep(30)
        res = bass_utils.run_bass_kernel_spmd(
            nc, in_maps, core_ids=list(range(NCORES)), trace=_trace
        )
    _CACHE["last_results"] = res
    nt = NTOK // TT
    outs = [
        r["outT"].reshape(nt, MC, 128, TT).transpose(0, 3, 1, 2).reshape(NTOK, D)
        for r in res.results
    ]
    return np.ascontiguousarray(np.concatenate(outs, axis=0), dtype=np.float32)


# revision 27
# speedup vs baseline: 1.0013x; 1.0013x over previous
"""MoE MLP (dense all-experts routing) Trainium2 Bass kernel.

Math (reference):
    g   = softmax(x @ gate_w + gate_b)            # [N, E]
    h   = relu(einsum("nd,edh->neh", x, w1) + b1) # [N, E, H]
    out = einsum("neh,ehd,ne->nd", h, w2, g)      # [N, D]

With E=64, H=16 (E*H = 1024 = D) this is two dense [1024,1024] matmuls plus a
small gate matmul.  Expert-hidden axis is reordered as eh' = h*64 + e
(h-major) so the gating multiply is a plain elementwise multiply of every
128-row tile of h^T by one shared [128, tok] tile of duplicated gate probs.

Layouts on device are feature-major (x^T, h^T, out^T); tokens are the matmul
moving (free) dimension.  Sharding: data-parallel over tokens, 4096 per core,
8 cores, no collectives.  Matmuls run in bf16 with fp32 PSUM accumulation.

Perf notes (vs the 255.5us previous version; PE floor for this decomposition
is ~225us of pure matmul streaming):
  - Gate logits col-tiled: even k -> partitions 0:64, odd k -> 64:128,
    concurrent pairs in the two array halves (4 PE slots per tile for 8 k
    chunks).  The two halves are combined WITHOUT a merge matmul:
    exp(a+b+gb) = exp(a)*exp(b+gb), so one scalar exp over all 128
    partitions (bias [0;gb]) followed by a partition-shift copy of the
    upper half and an elementwise multiply+dup (DVE reads may come from any
    partition window; writes can't straddle the 64-partition boundary).
    Saves one PE slot per tile and the lgsb round-trip.
  - Head: misc consts ride the fast sync ring FIRST (they gate the gate
    matmuls); x tile 0 is split 4+2 chunks on sync for progressive arrival;
    xb0 rides scalar ahead of w2 so gate pair 3 isn't stuck behind xa0.
    Warmup weights are memset on gpsimd (done ~6.5us) so the HAM warmup
    matmuls start right at PE preamble end (~7.2us).
  - 14 N=512 warmup matmuls cover the initial DMA wait and un-throttle the
    PE clock gate (4/8 -> 8/8) so real work starts warm.
  - Tail: the last tile's outputs avoid the gpsimd software-DGE queue
    (its end-of-kernel DRAIN polls ~3us behind the last packet); they ride
    sync, with the final block split sync/scalar so the teardown barrier
    isn't gated on a late software-queue drain.
  - DMA queues share HBM per-queue round-robin; the sync ring (~190GB/s)
    carries misc + x + w1 pairs 1-3 in exact consumption order; w1 pair 0
    rides the otherwise-idle gpsimd ring; w2 pairs ride scalar behind xb.
    The x pool is single-buffered so tile t+1's x DMA stays behind tile
    t's last read instead of being hoisted above the weight stream.
"""

import numpy as np
import ml_dtypes

N, D, E, H = 32768, 1024, 64, 16
EH = E * H  # 1024
NCORES = 8
NTOK = N // NCORES  # tokens per core
TT = 512            # token tile (one PSUM bank of fp32)
KC = D // 128       # 8 contraction chunks for D
MC = EH // 128      # 8 output row-tiles for EH (and for D in stage 2)
WARMUP = 13         # N=512 warmup matmuls covering the initial DMA wait

_CACHE = {}


def build(n_tok=NTOK):
    """Build + compile the per-core Bass kernel for n_tok tokens."""
    import concourse.bass as bass
    import concourse.mybir as mybir
    import concourse.tile as tile
    from concourse import bacc

    f32 = mybir.dt.float32
    bf16 = mybir.dt.bfloat16
    AF = mybir.ActivationFunctionType
    nt = n_tok // TT
    assert n_tok % TT == 0

    nc = bacc.Bacc("TRN2", target_bir_lowering=False, debug=False)

    KA = 6  # x-tile k-chunks in the first (big) half; KC-KA in the second
    xda = nc.dram_tensor("xda", [nt, 128, KA, TT], bf16, kind="ExternalInput")
    xdb = nc.dram_tensor("xdb", [nt, 128, KC - KA, TT], bf16, kind="ExternalInput")
    # all small consts packed into one tensor = one DMA:
    # cols [0:512] gate_w (k-major), [512] exp bias ([0]*64 || gate_b),
    # [513:521] b1 m-columns -- biases in bf16
    miscd = nc.dram_tensor("miscd", [128, 524], bf16, kind="ExternalInput")
    # weights in m-block PAIRS so each DMA is 4KB/partition (4KB descriptors
    # get full ring throughput; 2KB ones don't)
    w1d = nc.dram_tensor("w1d", [MC // 2, 128, 2, KC, 128], bf16, kind="ExternalInput")
    w2d = nc.dram_tensor("w2d", [MC // 2, 128, 2, KC, 128], bf16, kind="ExternalInput")
    outT = nc.dram_tensor("outT", [nt, MC, 128, TT], bf16, kind="ExternalOutput")

    with tile.TileContext(nc) as tc:
        with (
            tc.tile_pool(name="consts", bufs=1) as consts,
            tc.tile_pool(name="xp", bufs=1) as xp,
            tc.tile_pool(name="sp", bufs=2) as sp,
            tc.tile_pool(name="hp", bufs=4) as hp,
            tc.tile_pool(name="ps1", bufs=1, space=bass.MemorySpace.PSUM) as ps1,
            tc.tile_pool(name="ps2", bufs=3, space=bass.MemorySpace.PSUM) as ps2,
            tc.tile_pool(name="ps3", bufs=3, space=bass.MemorySpace.PSUM) as ps3,
        ):
            # --- delivery plan: ring FIFO is the priority mechanism.
            # sync ring: misc -> xa0 (split 4+2) -> w1 pairs 1-3 -> xa tiles
            #   and odd outputs.
            # scalar ring: xb0 -> w2 pairs -> xb tiles.
            # gpsimd ring: w1 pair 0, then even outputs (none for the last
            #   tile, so its software-DGE drain isn't on the tail path). ---
            # halfZ of 0.5 contracts both duplicated e2 halves over all 128
            # partitions (= sum over the 64 experts); full-row LDWEIGHTS
            # keeps pull-ahead pipelining (a 64-row Z would block it)
            halfZ = consts.tile([128, 128], bf16)
            nc.gpsimd.memset(halfZ[:], 0.5)
            # warmup weights memset on gpsimd: ready before the PE preamble
            # ends, unlike a vector memset (vector preamble ends ~7.4us)
            wsc = consts.tile([128, TT], bf16, tag="wsc")
            nc.gpsimd.memset(wsc[:], 0.0)

            # The head is HBM-bandwidth-bound (~300GB/s aggregate from
            # ~8.5us); delivery order = first-use order:
            #   sync (fast ~200GB/s ramp):  xa0 -> w1 m0 -> w1 pairs 1-3
            #   gpsimd (software DGE):      w1 m1
            #   scalar (slow ~80GB/s ramp): misc -> xb0 -> w2 pairs
            # CRITICAL: any PE idle gap over ~1us in the first ~25us makes
            # the HAM clock-gate RE-THROTTLE to 4/8 for a 3.4-6.8us window
            # (half-rate matmuls); the tile-0 schedule below keeps the PE
            # continuously fed (warmup fills bridge the sub-us waits).
            misc = consts.tile([128, 524], bf16)
            nc.scalar.dma_start(out=misc[:], in_=miscd[:])

            xa0 = xp.tile([128, KA, TT], bf16, tag="xa")
            nc.sync.dma_start(out=xa0[:], in_=xda[0])
            xb0 = xp.tile([128, KC - KA, TT], bf16, tag="xb")
            nc.scalar.dma_start(out=xb0[:], in_=xdb[0])

            w1_sb = consts.tile([128, MC, KC, 128], bf16)
            w2_sb = consts.tile([128, MC, KC, 128], bf16)
            nc.sync.dma_start(out=w1_sb[:, 0], in_=w1d[0][:, 0])
            nc.gpsimd.dma_start(out=w1_sb[:, 1], in_=w1d[0][:, 1])
            for p in range(1, MC // 2):
                nc.sync.dma_start(out=w1_sb[:, 2 * p:2 * p + 2], in_=w1d[p])
            for p in range(MC // 2):
                nc.scalar.dma_start(out=w2_sb[:, 2 * p:2 * p + 2], in_=w2d[p])

            def gw_k(k):
                return misc[:, k * 64:(k + 1) * 64]

            expbias = misc[:, 512:513]

            def b1_m(m):
                return misc[:, 513 + m:514 + m]

            # HAM warmup: dummy matmuls on a zeroed scratch tile fill the
            # initial weight/x DMA wait and un-throttle the PE clock gate
            # (4/8 -> 8/8) before real work arrives.  They use the zb slot
            # (first real use: the Z matmul, well after the gate) so the
            # gate's lg group has no dependency on them.
            wps = ps1.tile([128, TT], f32, tag="zb")
            for i in range(WARMUP):
                nc.tensor.matmul(wps[:], wsc[:, 0:128], wsc[:],
                                 start=(i == 0), stop=(i == WARMUP - 1))

            def wfill():
                # single filler matmul: bridges a sub-us data wait in the
                # tile-0 schedule so the PE never idles (idle > ~1us makes
                # the HAM clock-gate re-throttle to half rate)
                nc.tensor.matmul(wps[:], wsc[:, 0:128], wsc[:],
                                 start=True, stop=True)

            for t in range(nt):
                # x tile in two k-halves on separate HWDGE rings so each
                # tile lands as early as possible
                if t == 0:
                    xa, xb = xa0, xb0
                else:
                    xa = xp.tile([128, KA, TT], bf16, tag="xa")
                    nc.sync.dma_start(out=xa[:], in_=xda[t])
                    xb = xp.tile([128, KC - KA, TT], bf16, tag="xb")
                    nc.scalar.dma_start(out=xb[:], in_=xdb[t])

                def xk(k):
                    return xa[:, k, :] if k < KA else xb[:, k - KA, :]

                # --- gate logits, col-tiled: even k chunks accumulate into
                # partitions 0:64 (array cols 0-63), odd into 64:128 --
                # consecutive pairs run concurrently in the array halves.
                hg = sp.tile([128, MC, TT], bf16, tag="hg")
                h_tiles = []

                def stage1_mm(m, ks=range(KC), hps=None):
                    if hps is None:
                        hps = ps2.tile([128, TT], f32, tag="hps")
                    for k in ks:
                        nc.tensor.matmul(
                            hps[:], w1_sb[:, m, k, :], xk(k),
                            start=(k == 0), stop=(k == KC - 1),
                            skip_group_check=True,
                        )
                    return hps

                def stage1_act(m, hps):
                    h = hp.tile([128, TT], bf16, tag="h")
                    nc.scalar.activation(
                        h[:], hps[:], AF.Relu, bias=b1_m(m), scale=1.0
                    )
                    h_tiles.append((m, h))

                def stage1(m):
                    stage1_act(m, stage1_mm(m))

                lg = ps1.tile([128, TT], f32, tag="lg")

                def gate_pair(j):
                    nc.tensor.matmul(
                        lg[0:64, :], gw_k(2 * j), xk(2 * j),
                        start=(j == 0), stop=(j == 3), skip_group_check=True,
                    )
                    nc.tensor.matmul(
                        lg[64:128, :], gw_k(2 * j + 1), xk(2 * j + 1),
                        start=(j == 0), stop=(j == 3), skip_group_check=True,
                    )

                if t == 0:
                    # tile 0 in data-arrival order, gap-free: pairs 0-2
                    # (misc+xa0 ~12.5us), m0-m2 xa chunks as w1 m-blocks
                    # land (sync m0 ~13.8, gpsimd m1 ~15.5, sync pair1
                    # ~16.4), pair 3 once xb0 lands on scalar (~17), then
                    # the deferred k6/k7 completions.  wfill() bridges the
                    # two sub-us waits.
                    gate_pair(0)
                    gate_pair(1)
                    gate_pair(2)
                    wfill()
                    hps0 = stage1_mm(0, range(KA))
                    wfill()
                    hps1 = stage1_mm(1, range(KA))
                    hps2 = stage1_mm(2, range(KA))
                    gate_pair(3)
                    stage1_mm(0, range(KA, KC), hps0)
                    stage1_mm(1, range(KA, KC), hps1)
                    stage1_mm(2, range(KA, KC), hps2)
                else:
                    for j in range(4):
                        gate_pair(j)
                    hps0 = stage1_mm(0)

                # duplicated gate probs without a merge matmul:
                # e2[p] = exp(a_p)*exp(b_p+gb) for p%64 halves; exp runs
                # straight off the lg PSUM, the upper half is shift-copied
                # down, multiplied, and the product shift-copied back up.
                e_all = sp.tile([128, TT], bf16, tag="eall")
                nc.scalar.activation(e_all[:], lg[:], AF.Exp,
                                     bias=expbias, scale=1.0)
                et = sp.tile([64, TT], bf16, tag="etmp")
                nc.vector.tensor_copy(et[:], e_all[64:128, :])
                e2 = sp.tile([128, TT], bf16, tag="e2")
                nc.vector.tensor_mul(e2[0:64, :], e_all[0:64, :], et[:])
                nc.vector.tensor_copy(e2[64:128, :], e2[0:64, :])
                stage1_act(0, hps0)

                if t == 0:
                    stage1_act(1, hps1)
                else:
                    stage1(1)

                # Z = sum of exp over the 64 experts, broadcast to all 128
                # partitions (0.5 * both duplicated halves), then g2 = e2/Z
                zb = ps1.tile([128, TT], f32, tag="zb")
                nc.tensor.matmul(zb[:], halfZ[:], e2[:], start=True, stop=True)

                if t == 0:
                    stage1_act(2, hps2)
                else:
                    stage1(2)

                rzb = sp.tile([128, TT], f32, tag="rzb")
                nc.vector.reciprocal_approx_fast(rzb[:], zb[:])
                g2 = sp.tile([128, TT], bf16, tag="g2")
                nc.vector.tensor_mul(g2[:], e2[:], rzb[:])

                for m, h in h_tiles:
                    nc.vector.tensor_mul(hg[:, m, :], h[:], g2[:])
                for m in range(3, MC):
                    stage1(m)
                    _, h = h_tiles[-1]
                    nc.vector.tensor_mul(hg[:, m, :], h[:], g2[:])

                # --- stage 2: out^T tiles ---
                last = t == nt - 1
                for m2 in range(MC):
                    ops = ps3.tile([128, TT], f32, tag="ops")
                    for k in range(MC):
                        nc.tensor.matmul(
                            ops[:], w2_sb[:, m2, k, :], hg[:, k, :],
                            start=(k == 0), stop=(k == MC - 1),
                        )
                    osb = hp.tile([128, TT], bf16, tag="osb")
                    if last and m2 == MC - 1:
                        # final output block: cast halves on two engines and
                        # DMA on two hardware rings in parallel so the tail
                        # drains sooner (gpsimd's software queue would add
                        # ~3us of end-of-kernel drain polling)
                        nc.vector.tensor_copy(osb[:, 0:TT // 2], ops[:, 0:TT // 2])
                        nc.scalar.copy(osb[:, TT // 2:], ops[:, TT // 2:])
                        nc.sync.dma_start(
                            out=outT[t, m2][:, 0:TT // 2], in_=osb[:, 0:TT // 2])
                        nc.scalar.dma_start(
                            out=outT[t, m2][:, TT // 2:], in_=osb[:, TT // 2:])
                        continue
                    nc.vector.tensor_copy(osb[:], ops[:])
                    # alternate output rings: halves the gpsimd ring load
                    # and lets the last tile's DMAs drain in parallel; the
                    # last tile stays entirely off the gpsimd ring
                    if m2 % 2 == 1 or last:
                        nc.sync.dma_start(out=outT[t, m2], in_=osb[:])
                    else:
                        nc.gpsimd.dma_start(out=outT[t, m2], in_=osb[:])

    nc.compile()
    return nc


def host_prep(x, gate_w, gate_b, w1, b1, w2):
    bf = ml_dtypes.bfloat16
    nt = NTOK // TT
    KA = 6
    xb = x.astype(bf)
    x_shards = []
    for c in range(NCORES):
        xt = (xb[c * NTOK:(c + 1) * NTOK]
              .reshape(nt, TT, KC, 128).transpose(0, 3, 2, 1))
        x_shards.append((np.ascontiguousarray(xt[:, :, :KA]),
                         np.ascontiguousarray(xt[:, :, KA:])))
    miscd = np.zeros((128, 524), dtype=bf)
    miscd[:, 0:512] = (
        gate_w.astype(bf).reshape(KC, 128, 64).transpose(1, 0, 2)
        .reshape(128, 512))
    # exp bias: upper half gets gate_b, lower half 0 (the product of the
    # two exp halves then carries exp(gate_b) exactly once)
    miscd[64:128, 512] = gate_b.astype(bf)
    # eh' = h*64 + e ordering
    miscd[:, 513:521] = b1.T.reshape(EH).astype(bf).reshape(MC, 128).T
    w1d = np.ascontiguousarray(
        w1.transpose(1, 2, 0).reshape(D, EH).astype(bf)
        .reshape(KC, 128, MC // 2, 2, 128).transpose(2, 1, 3, 0, 4))
    w2d = np.ascontiguousarray(
        w2.transpose(1, 0, 2).reshape(EH, D).astype(bf)
        .reshape(KC, 128, MC // 2, 2, 128).transpose(2, 1, 3, 0, 4))
    common = {"miscd": miscd, "w1d": w1d, "w2d": w2d}
    return x_shards, common


def kernel(x, gate_w, gate_b, w1, b1, w2, _trace=False):
    import concourse.bass_utils as bass_utils

    x = np.asarray(x, dtype=np.float32)
    gate_w = np.asarray(gate_w, dtype=np.float32)
    gate_b = np.asarray(gate_b, dtype=np.float32)
    w1 = np.asarray(w1, dtype=np.float32)
    b1 = np.asarray(b1, dtype=np.float32)
    w2 = np.asarray(w2, dtype=np.float32)

    if "nc" not in _CACHE:
        _CACHE["nc"] = build(NTOK)
    nc = _CACHE["nc"]

    x_shards, common = host_prep(x, gate_w, gate_b, w1, b1, w2)
    in_maps = [dict(common, xda=x_shards[c][0], xdb=x_shards[c][1])
               for c in range(NCORES)]
    try:
        res = bass_utils.run_bass_kernel_spmd(
            nc, in_maps, core_ids=list(range(NCORES)), trace=_trace
        )
    except Exception:
        # transient device states (e.g. NRT_EXEC_UNIT_UNRECOVERABLE after a
        # wedged prior run) usually clear after a pause; retry once
        import time
        time.sleep(30)
        res = bass_utils.run_bass_kernel_spmd(
            nc, in_maps, core_ids=list(range(NCORES)), trace=_trace
        )
    _CACHE["last_results"] = res
    nt = NTOK // TT
    outs = [
        r["outT"].reshape(nt, MC, 128, TT).transpose(0, 3, 1, 2).reshape(NTOK, D)
        for r in res.results
    ]
    return np.ascontiguousarray(np.concatenate(outs, axis=0), dtype=np.float32)
